# revision 46
# baseline (speedup 1.0000x reference)
"""Causal self-attention (B=4, T=2048, C=1024, H=16, D=64) on 8 TRN2 cores.

Sharding: core i = (batch b=i//2, head-group g=i%2 of 8 heads).
Each core runs the full pipeline for its (b, g) shard with zero
cross-core communication; the row-parallel out_proj partial sums of the
two head-groups of a batch are added on the host during unsharding.

Compute dtype: bfloat16 operands, fp32 PSUM accumulation (full PE rate).
Host converts inputs to bf16; the final output is fp32.

Per-core dataflow:
  phase 1: qkv projection from xT (feature-major x), producing
           qT/kT [512,2048] feature-major and v [2048,512] time-major,
           spilled to DRAM scratch (bf16).
  phase 2: per head: scores^T tiles [t2,t1] = kT_h' @ qT_h (K=64),
           exp on ScalarE (no max subtraction -- scores are O(1)),
           causal triangular mask on diagonal 128x128 blocks only,
           PV matmul with lhsT=[v_h | ones] so PSUM row 64 accumulates
           the softmax denominator; normalize on the way to SBUF.
  phase 3: out_proj partial = attn_outT' @ w_out rows for this group.
"""

import os
import sys

for _p in (
    "/root/.axon_site",
    "/root/.axon_site/_ro/trn_rl_repo",
    "/root/.axon_site/_ro/pypackages",
    "/opt/trn_rl_repo",
):
    if os.path.isdir(_p) and _p not in sys.path:
        sys.path.append(_p)

import numpy as np
import ml_dtypes

import concourse.bass as bass
import concourse.bacc as bacc
import concourse.mybir as mybir
from concourse import tile
from concourse.bass_utils import run_bass_kernel_spmd

BF16NP = ml_dtypes.bfloat16

B, T, C, H, D = 4, 2048, 1024, 16, 64
HPC = 8            # heads per core
GF = HPC * D       # 512: feature width of one head-group
NCORES = 8
KC = C // 128      # 8 contraction tiles over C
NT = T // 128      # 16 time tiles of 128
TS = 512           # t1 slice width
NTS = T // TS      # 4 t1 slices

F32 = mybir.dt.float32
F16 = mybir.dt.float16
BF16 = mybir.dt.bfloat16
AF = mybir.ActivationFunctionType


def build_nc() -> bass.Bass:
    nc = bacc.Bacc()

    xT = nc.declare_dram_parameter("xT", [C, T], BF16, isOutput=False)
    wq = nc.declare_dram_parameter("wq", [C, GF], BF16, isOutput=False)
    wk = nc.declare_dram_parameter("wk", [C, GF], BF16, isOutput=False)
    wv = nc.declare_dram_parameter("wv", [C, GF], BF16, isOutput=False)
    wo = nc.declare_dram_parameter("wo", [GF, C], BF16, isOutput=False)
    bqc = nc.declare_dram_parameter("bqc", [128, 4], F32, isOutput=False)
    bkc = nc.declare_dram_parameter("bkc", [128, 4], F32, isOutput=False)
    bv = nc.declare_dram_parameter("bv", [1, GF], BF16, isOutput=False)
    tri01 = nc.declare_dram_parameter("tri01", [128, 128], BF16, isOutput=False)
    out = nc.declare_dram_parameter("out", [T, C], F16, isOutput=True)

    with tile.TileContext(nc) as tc:
        with (
            tc.tile_pool(name="dram", bufs=1, space="DRAM") as dpool,
            tc.tile_pool(name="consts", bufs=1) as cpool,
            tc.tile_pool(name="apool", bufs=1) as apool,
        ):
            # v resident in SBUF: [t2-part, t2-tile, head, 64 v + 1 ones]
            v_sb = apool.tile([128, NT, HPC, 65], BF16, tag="vsb")
            nc.vector.memset(v_sb[:, :, :, 64:65], 1.0)
            # attention outputs stay resident in SBUF, feature-major
            aP = [apool.tile([128, T], BF16, tag=f"aP{f}", name=f"aP{f}") for f in range(4)]

            # const loads go on the Pool trigger queue, after wk (see below):
            # the Sync queue must reach the f=0 critical-path loads first
            tri_sb = cpool.tile([128, 128], BF16)
            bqc_sb = cpool.tile([128, 4], F32, tag="bqc")
            bkc_sb = cpool.tile([128, 4], F32, tag="bkc")
            bv_sb = cpool.tile([1, GF], BF16, tag="bv")
            bvb_sb = cpool.tile([128, GF], BF16, tag="bvb")

            # ------- phases 1+2 interleaved: qkv projection + attention -------
            with (
                tc.tile_pool(name="p1", bufs=1) as p1,
                tc.tile_pool(name="p1s", bufs=16) as p1s,
                tc.tile_pool(name="p2", bufs=2) as p2,
                tc.tile_pool(name="pE", bufs=10) as pE,
                tc.tile_pool(name="pN", bufs=2) as pN,
                tc.tile_pool(name="p3s", bufs=6) as p3s,
                tc.tile_pool(name="pp1", bufs=2, space="PSUM") as pp1,
                tc.tile_pool(name="ps2", bufs=1, space="PSUM") as ps2,
                tc.tile_pool(name="pso", bufs=2, space="PSUM") as pso,
            ):
                # DMA priority order over two trigger queues (Sync + Pool):
                # the f=0 q/k projection's inputs first, wo last.
                xT_sb = p1.tile([128, KC, T], BF16)
                wq_sb = p1.tile([128, KC, GF], BF16, tag="wq")
                wk_sb = p1.tile([128, KC, GF], BF16, tag="wk")
                wv_sb = p1.tile([128, KC, GF], BF16, tag="wv")
                wo_sb = p1.tile([128, 4, C], BF16, tag="wo")
                for k in range(KC):
                    ksl = slice(k * 128, (k + 1) * 128)
                    nc.sync.dma_start(wq_sb[:, k, :], wq[ksl, :])
                    nc.sync.dma_start(xT_sb[:, k, :], xT[ksl, :])
                for k in range(KC):
                    ksl = slice(k * 128, (k + 1) * 128)
                    nc.gpsimd.dma_start(wk_sb[:, k, :], wk[ksl, :])
                nc.gpsimd.dma_start(bqc_sb[:, :], bqc[:, :])
                nc.gpsimd.dma_start(bkc_sb[:, :], bkc[:, :])
                nc.gpsimd.dma_start(bv_sb[:, :], bv[:, :])
                nc.gpsimd.dma_start(tri_sb[:, :], tri01[:, :])
                # bias of v broadcast to all partitions once; folded into the
                # PSUM->SBUF copy of v (replaces the ones/bv bias matmul).
                # Also acts as a throttle: the wv/wo triggers behind it fire
                # only after bv lands, keeping early HBM bandwidth for wq/xT.
                nc.gpsimd.partition_broadcast(bvb_sb[:, :], bv_sb[:, :])
                for k in range(KC):
                    ksl = slice(k * 128, (k + 1) * 128)
                    nc.gpsimd.dma_start(wv_sb[:, k, :], wv[ksl, :])
                nc.gpsimd.dma_start(
                    wo_sb[:, :, :], wo[:, :].rearrange("(n p) c -> p n c", p=128)
                )

                # ---- projection chain emitters (each ~8 matmuls of filler
                # work for the Tensor queue, used to absorb exp-wait stalls
                # in the Scalar-bound attention phases) ----
                fqk = {}

                def alloc_qk(f):
                    fqk[f] = (
                        p2.tile([128, T], BF16, tag="qp", name=f"qp{f}"),
                        p2.tile([128, T], BF16, tag="kp", name=f"kp{f}"),
                    )

                qk_done = set()

                def emit_qk(f, ts):
                    # q+k projection chains for head-pair f, t1-slice ts
                    if (f, ts) in qk_done:
                        return
                    qk_done.add((f, ts))
                    qpf, kpf = fqk[f]
                    for w_sb, b_sb, dst in (
                        (wq_sb, bqc_sb, qpf),
                        (wk_sb, bkc_sb, kpf),
                    ):
                        acc = pp1.tile([128, TS], F32, tag="acc")
                        for k in range(KC):
                            nc.tensor.matmul(
                                acc[:, :],
                                w_sb[:, k, f * 128 : (f + 1) * 128],
                                xT_sb[:, k, ts * TS : (ts + 1) * TS],
                                start=(k == 0),
                                stop=(k == KC - 1),
                            )
                        nc.vector.tensor_scalar_add(
                            dst[:, ts * TS : (ts + 1) * TS],
                            acc[:, :],
                            b_sb[:, f : f + 1],
                        )

                n_vdone = [0]

                def v_chain():
                    t = n_vdone[0]
                    n_vdone[0] += 1
                    acc = pp1.tile([128, GF], F32, tag="acc")
                    for k in range(KC):
                        nc.tensor.matmul(
                            acc[:, :],
                            xT_sb[:, k, t * 128 : (t + 1) * 128],
                            wv_sb[:, k, :],
                            start=(k == 0),
                            stop=(k == KC - 1),
                        )
                    nc.vector.tensor_tensor(
                        v_sb[:, t, :, 0:64],
                        acc[:, :].rearrange("p (h d) -> p h d", h=HPC),
                        bvb_sb[:, :].rearrange("p (h d) -> p h d", h=HPC),
                        op=mybir.AluOpType.add,
                    )

                alloc_qk(0)
                for f in range(4):
                    qp, kp = fqk[f]
                    if f < 3:
                        alloc_qk(f + 1)
                    # filler queue for this head-pair's attention phase:
                    # v chains (f=0; front so PV deps resolve), then next
                    # head-pair's q/k projection chains
                    fillers = []
                    if f == 0:
                        fillers += [("v", v_chain)] * NT
                    if f < 3:
                        fillers += [
                            ("qk", (lambda ff=f + 1, ts=ts: emit_qk(ff, ts)))
                            for ts in range(NTS)
                        ]
                    NB = sum(2 * t + 2 for t in range(NTS))
                    gi = [0]

                    def drain_to_target():
                        # keep pace: finish all fillers by the last batch
                        gi[0] += 1
                        remaining = max(NB - gi[0], 1)
                        n = (len(fillers) + remaining - 1) // remaining
                        for _ in range(min(n, len(fillers))):
                            fillers.pop(0)[1]()

                    def ensure_v(t2max):
                        # pop v-chain fillers (queue front) until key block
                        # t2max is projected; a PV emitted before its v chain
                        # would deadlock the in-order Tensor queue
                        while n_vdone[0] <= t2max:
                            kind, fn = fillers.pop(0)
                            assert kind == "v"
                            fn()

                    for t1i in range(NTS):
                        emit_qk(f, t1i)
                        t1s = t1i * TS
                        nfull = t1s // 128
                        o_ps = [
                            pso.tile([65, TS], F32, tag="outps", name=f"ops{r}")
                            for r in range(2)
                        ]
                        nmm = [0, 0]
                        batches = [("full", t2p) for t2p in range(nfull // 2)]
                        batches += [("diag", 0), ("diag", 1)]

                        def emit_scores(b, r):
                            kind, idx = b
                            s_ps = ps2.tile(
                                [128, 1024], F32, tag=f"sps{r}", name=f"sps{r}"
                            )
                            if kind == "full":
                                for j in range(2):
                                    t2 = 2 * idx + j
                                    nc.tensor.matmul(
                                        s_ps[:, j * TS : (j + 1) * TS],
                                        kp[r * 64 : (r + 1) * 64, t2 * 128 : (t2 + 1) * 128],
                                        qp[r * 64 : (r + 1) * 64, t1s : t1s + TS],
                                        start=True,
                                        stop=True,
                                    )
                            else:
                                ds = (0, 1) if idx == 0 else (2, 3)
                                offs = (0, TS) if idx == 0 else (0, 256)
                                for d, off in zip(ds, offs):
                                    t2 = nfull + d
                                    nd = TS - 128 * d
                                    nc.tensor.matmul(
                                        s_ps[:, off : off + nd],
                                        kp[r * 64 : (r + 1) * 64, t2 * 128 : (t2 + 1) * 128],
                                        qp[r * 64 : (r + 1) * 64, t1s + 128 * d : t1s + TS],
                                        start=True,
                                        stop=True,
                                    )
                            return s_ps

                        def emit_exp(b, r, s_ps):
                            kind, idx = b
                            E = pE.tile([128, 1024], BF16, tag="E", name=f"E{r}")
                            if kind == "full":
                                nc.scalar.activation(E[:, :], s_ps[:, :], AF.Exp)
                            else:
                                width = 896 if idx == 0 else 384
                                nc.scalar.activation(
                                    E[:, 0:width], s_ps[:, 0:width], AF.Exp
                                )
                                # causal mask: zero the upper triangle of the
                                # two diagonal 128x128 blocks post-exp (bf16
                                # on DVE, off the PSUM critical path)
                                for off in (0, TS) if idx == 0 else (0, 256):
                                    nc.vector.tensor_mul(
                                        E[:, off : off + 128],
                                        E[:, off : off + 128],
                                        tri_sb[:, :],
                                    )
                            return E

                        def emit_pv(b, r, E):
                            kind, idx = b
                            if kind == "full":
                                for j in range(2):
                                    t2 = 2 * idx + j
                                    nc.tensor.matmul(
                                        o_ps[r][:, :],
                                        v_sb[:, t2, 2 * f + r, :],
                                        E[:, j * TS : (j + 1) * TS],
                                        start=(nmm[r] == 0),
                                        stop=False,
                                    )
                                    nmm[r] += 1
                            else:
                                ds = (0, 1) if idx == 0 else (2, 3)
                                offs = (0, TS) if idx == 0 else (0, 256)
                                for d, off in zip(ds, offs):
                                    t2 = nfull + d
                                    nd = TS - 128 * d
                                    nc.tensor.matmul(
                                        o_ps[r][:, 128 * d : TS],
                                        v_sb[:, t2, 2 * f + r, :],
                                        E[:, off : off + nd],
                                        start=(nmm[r] == 0),
                                        stop=(d == 3),
                                    )
                                    nmm[r] += 1

                        def t2max(b):
                            kind, idx = b
                            if kind == "full":
                                return 2 * idx + 1
                            return nfull + (1 if idx == 0 else 3)

                        # software pipeline: exps first on Scalar; on the
                        # Tensor queue PVs (E inputs long ready) and filler
                        # projection chains go ahead of scores(i+1) so the
                        # queue never idles behind a scores matmul waiting
                        # for exp(i) to free its PSUM bank
                        sps_cur = [emit_scores(batches[0], r) for r in range(2)]
                        pvq = []
                        for i, b in enumerate(batches):
                            Es = [emit_exp(b, r, sps_cur[r]) for r in range(2)]
                            if len(pvq) >= 1:
                                pb, pEs = pvq.pop(0)
                                ensure_v(t2max(pb))
                                for r in range(2):
                                    emit_pv(pb, r, pEs[r])
                            sps_nxt = [None, None]
                            if i + 1 < len(batches):
                                sps_nxt = [
                                    emit_scores(batches[i + 1], r) for r in range(2)
                                ]
                            # fillers go BEHIND scores(i+1): scores re-enter
                            # the PSUM round-trip immediately; fillers only
                            # soak up whatever Tensor idle remains
                            drain_to_target()
                            pvq.append((b, Es))
                            sps_cur = sps_nxt
                        for pb, pEs in pvq:
                            ensure_v(t2max(pb))
                            for r in range(2):
                                emit_pv(pb, r, pEs[r])

                        # normalize rows 0-63 by row 64 (softmax denominator):
                        # reciprocal on the [1,512] row, then broadcast
                        for r in range(2):
                            denom = pN.tile([1, TS], F32, tag="dn", name=f"dn{r}")
                            nc.vector.tensor_copy(denom[:, :], o_ps[r][64:65, :])
                            bc_sb = pN.tile([64, TS], F32, tag="bc", name=f"bc{r}")
                            nc.gpsimd.partition_broadcast(bc_sb[:, :], denom[:, :])
                            rc_sb = pN.tile([64, TS], F32, tag="rc", name=f"rc{r}")
                            nc.vector.reciprocal_approx_fast(rc_sb[:, :], bc_sb[:, :])
                            nc.vector.tensor_mul(
                                aP[f][r * 64 : (r + 1) * 64, t1s : t1s + TS],
                                o_ps[r][0:64, :],
                                rc_sb[:, :],
                            )
                        if f == 3:
                            # all heads done for this t1 range: output
                            # projection chunks become fillers for the next
                            # slice's attention (last slice drains at the end)
                            def oproj_chunk(t, n):
                                acc3 = pp1.tile([128, TS], F32, tag="acc")
                                for ff in range(4):
                                    nc.tensor.matmul(
                                        acc3[:, :],
                                        aP[ff][:, t * 128 : (t + 1) * 128],
                                        wo_sb[:, ff, n * TS : (n + 1) * TS],
                                        start=(ff == 0),
                                        stop=(ff == 3),
                                    )
                                stg = p3s.tile([128, TS], F16, tag="stg3")
                                if (t + n) % 2 == 0:
                                    nc.vector.tensor_copy(stg[:, :], acc3[:, :])
                                else:
                                    nc.scalar.copy(stg[:, :], acc3[:, :])
                                nc.gpsimd.dma_start(
                                    out[
                                        t * 128 : (t + 1) * 128,
                                        n * TS : (n + 1) * TS,
                                    ],
                                    stg[:, :],
                                )

                            for t in range(t1s // 128, t1s // 128 + 4):
                                for n in range(2):
                                    fillers.append(
                                        ("op", (lambda tt=t, nn=n: oproj_chunk(tt, nn)))
                                    )

                    while fillers:
                        fillers.pop(0)[1]()

    nc.finalize()
    return nc


def make_in_maps(x, w_qkv, b_qkv, w_out, b_out):
    x = np.asarray(x, dtype=np.float32)
    w_qkv = np.asarray(w_qkv, dtype=np.float32)
    b_qkv = np.asarray(b_qkv, dtype=np.float32)
    w_out = np.asarray(w_out, dtype=np.float32)

    def bf(a):
        return np.ascontiguousarray(a).astype(BF16NP)

    scale = 1.0 / np.sqrt(D)
    # multiplicative causal mask for diagonal blocks, applied to exp(scores):
    # rows = keys, cols = queries; keep where query >= key (upper triangle)
    tri01 = np.triu(np.ones((128, 128), dtype=np.float32)).astype(BF16NP)
    in_maps = []
    for core in range(NCORES):
        b, g = core // 2, core % 2
        sl = slice(g * GF, (g + 1) * GF)
        bq = (b_qkv[sl] * scale).reshape(4, 128).T  # [128, 4] per-feat col
        bk = b_qkv[C + g * GF : C + (g + 1) * GF].reshape(4, 128).T
        in_maps.append(
            {
                "xT": bf(x[b].T),
                "wq": bf(w_qkv[:, sl] * scale),
                "wk": bf(w_qkv[:, C + g * GF : C + (g + 1) * GF]),
                "wv": bf(w_qkv[:, 2 * C + g * GF : 2 * C + (g + 1) * GF]),
                "wo": bf(w_out[sl, :]),
                "bqc": np.ascontiguousarray(bq, dtype=np.float32),
                "bkc": np.ascontiguousarray(bk, dtype=np.float32),
                "bv": bf(b_qkv[2 * C + g * GF : 2 * C + (g + 1) * GF].reshape(1, GF)),
                "tri01": tri01,
            }
        )
    return in_maps


_NC_CACHE = {}


def run(inputs: dict, trace: bool = False):
    """Compile (cached) + run on 8 cores. Returns (full_output, BassKernelResults)."""
    if "nc" not in _NC_CACHE:
        _NC_CACHE["nc"] = build_nc()
    nc = _NC_CACHE["nc"]
    in_maps = make_in_maps(**inputs)
    res = run_bass_kernel_spmd(
        nc, in_maps, core_ids=list(range(NCORES)), trace=trace
    )
    outs = [np.asarray(m["out"], dtype=np.float32) for m in res.results]
    full = np.stack([outs[2 * b] + outs[2 * b + 1] for b in range(B)], axis=0)
    full += np.asarray(inputs["b_out"], dtype=np.float32)
    return full, res


def kernel(**inputs) -> np.ndarray:
    full, _ = run(inputs, trace=False)
    return full



# revision 47
# speedup vs baseline: 1.0527x; 1.0527x over previous
"""Causal self-attention (B=4, T=2048, C=1024, H=16, D=64) on 8 TRN2 cores.

Sharding: core i = (batch b=i//2, head-group g=i%2 of 8 heads).
Each core runs the full pipeline for its (b, g) shard with zero
cross-core communication; the row-parallel out_proj partial sums of the
two head-groups of a batch are added on the host during unsharding.

Compute dtype: bfloat16 operands, fp32 PSUM accumulation (full PE rate).
Host converts inputs to bf16; the final output is fp32.

Per-core dataflow:
  phase 1: qkv projection from xT (feature-major x), producing
           qT/kT [512,2048] feature-major and v [2048,512] time-major,
           spilled to DRAM scratch (bf16).
  phase 2: per head: scores^T tiles [t2,t1] = kT_h' @ qT_h (K=64),
           exp on ScalarE (no max subtraction -- scores are O(1)),
           causal triangular mask on diagonal 128x128 blocks only,
           PV matmul with lhsT=[v_h | ones] so PSUM row 64 accumulates
           the softmax denominator; normalize on the way to SBUF.
  phase 3: out_proj partial = attn_outT' @ w_out rows for this group.
"""

import os
import sys

for _p in (
    "/root/.axon_site",
    "/root/.axon_site/_ro/trn_rl_repo",
    "/root/.axon_site/_ro/pypackages",
    "/opt/trn_rl_repo",
):
    if os.path.isdir(_p) and _p not in sys.path:
        sys.path.append(_p)

import numpy as np
import ml_dtypes

import concourse.bass as bass
import concourse.bacc as bacc
import concourse.mybir as mybir
from concourse import tile
from concourse.bass_utils import run_bass_kernel_spmd

BF16NP = ml_dtypes.bfloat16

B, T, C, H, D = 4, 2048, 1024, 16, 64
HPC = 8            # heads per core
GF = HPC * D       # 512: feature width of one head-group
NCORES = 8
KC = C // 128      # 8 contraction tiles over C
NT = T // 128      # 16 time tiles of 128
TS = 512           # t1 slice width
NTS = T // TS      # 4 t1 slices

F32 = mybir.dt.float32
F16 = mybir.dt.float16
BF16 = mybir.dt.bfloat16
AF = mybir.ActivationFunctionType


def build_nc() -> bass.Bass:
    nc = bacc.Bacc()

    xT = nc.declare_dram_parameter("xT", [C, T], BF16, isOutput=False)
    wq = nc.declare_dram_parameter("wq", [C, GF], BF16, isOutput=False)
    wk = nc.declare_dram_parameter("wk", [C, GF], BF16, isOutput=False)
    wv = nc.declare_dram_parameter("wv", [C, GF], BF16, isOutput=False)
    wo = nc.declare_dram_parameter("wo", [GF, C], BF16, isOutput=False)
    bqc = nc.declare_dram_parameter("bqc", [128, 4], F32, isOutput=False)
    bkc = nc.declare_dram_parameter("bkc", [128, 4], F32, isOutput=False)
    bv = nc.declare_dram_parameter("bv", [1, GF], BF16, isOutput=False)
    tri01 = nc.declare_dram_parameter("tri01", [128, 128], BF16, isOutput=False)
    out = nc.declare_dram_parameter("out", [T, C], F16, isOutput=True)

    with tile.TileContext(nc) as tc:
        with (
            tc.tile_pool(name="dram", bufs=1, space="DRAM") as dpool,
            tc.tile_pool(name="consts", bufs=1) as cpool,
            tc.tile_pool(name="apool", bufs=1) as apool,
        ):
            # v resident in SBUF: [t2-part, t2-tile, head, 64 v + 1 ones]
            v_sb = apool.tile([128, NT, HPC, 65], BF16, tag="vsb")
            nc.vector.memset(v_sb[:, :, :, 64:65], 1.0)
            # attention outputs stay resident in SBUF, feature-major
            aP = [apool.tile([128, T], BF16, tag=f"aP{f}", name=f"aP{f}") for f in range(4)]

            # const loads go on the Pool trigger queue, after wk (see below):
            # the Sync queue must reach the f=0 critical-path loads first
            tri_sb = cpool.tile([128, 128], BF16)
            bqc_sb = cpool.tile([128, 4], F32, tag="bqc")
            bkc_sb = cpool.tile([128, 4], F32, tag="bkc")
            bv_sb = cpool.tile([1, GF], BF16, tag="bv")
            bvb_sb = cpool.tile([128, GF], BF16, tag="bvb")

            # ------- phases 1+2 interleaved: qkv projection + attention -------
            with (
                tc.tile_pool(name="p1", bufs=1) as p1,
                tc.tile_pool(name="p1s", bufs=16) as p1s,
                tc.tile_pool(name="p2", bufs=2) as p2,
                tc.tile_pool(name="pE", bufs=10) as pE,
                tc.tile_pool(name="pN", bufs=2) as pN,
                tc.tile_pool(name="p3s", bufs=6) as p3s,
                tc.tile_pool(name="pp1", bufs=2, space="PSUM") as pp1,
                tc.tile_pool(name="ps2", bufs=1, space="PSUM") as ps2,
                tc.tile_pool(name="pso", bufs=2, space="PSUM") as pso,
            ):
                # DMA priority order over two trigger queues (Sync + Pool):
                # the f=0 q/k projection's inputs first, wo last.
                xT_sb = p1.tile([128, KC, T], BF16)
                wq_sb = p1.tile([128, KC, GF], BF16, tag="wq")
                wk_sb = p1.tile([128, KC, GF], BF16, tag="wk")
                wv_sb = p1.tile([128, KC, GF], BF16, tag="wv")
                wo_sb = p1.tile([128, 4, C], BF16, tag="wo")
                for k in range(KC):
                    ksl = slice(k * 128, (k + 1) * 128)
                    nc.sync.dma_start(wq_sb[:, k, :], wq[ksl, :])
                    nc.sync.dma_start(xT_sb[:, k, :], xT[ksl, :])
                for k in range(KC):
                    ksl = slice(k * 128, (k + 1) * 128)
                    nc.gpsimd.dma_start(wk_sb[:, k, :], wk[ksl, :])
                nc.gpsimd.dma_start(bqc_sb[:, :], bqc[:, :])
                nc.gpsimd.dma_start(bkc_sb[:, :], bkc[:, :])
                nc.gpsimd.dma_start(bv_sb[:, :], bv[:, :])
                nc.gpsimd.dma_start(tri_sb[:, :], tri01[:, :])
                # bias of v broadcast to all partitions once; folded into the
                # PSUM->SBUF copy of v (replaces the ones/bv bias matmul).
                # Also acts as a throttle: the wv/wo triggers behind it fire
                # only after bv lands, keeping early HBM bandwidth for wq/xT.
                nc.gpsimd.partition_broadcast(bvb_sb[:, :], bv_sb[:, :])
                for k in range(KC):
                    ksl = slice(k * 128, (k + 1) * 128)
                    nc.gpsimd.dma_start(wv_sb[:, k, :], wv[ksl, :])
                nc.gpsimd.dma_start(
                    wo_sb[:, :, :], wo[:, :].rearrange("(n p) c -> p n c", p=128)
                )

                # ---- projection chain emitters (each ~8 matmuls of filler
                # work for the Tensor queue, used to absorb exp-wait stalls
                # in the Scalar-bound attention phases) ----
                fqk = {}

                def alloc_qk(f):
                    fqk[f] = (
                        p2.tile([128, T], BF16, tag="qp", name=f"qp{f}"),
                        p2.tile([128, T], BF16, tag="kp", name=f"kp{f}"),
                    )

                qk_done = set()

                def emit_qk(f, ts):
                    # q+k projection chains for head-pair f, t1-slice ts
                    if (f, ts) in qk_done:
                        return
                    qk_done.add((f, ts))
                    qpf, kpf = fqk[f]
                    for w_sb, b_sb, dst in (
                        (wq_sb, bqc_sb, qpf),
                        (wk_sb, bkc_sb, kpf),
                    ):
                        acc = pp1.tile([128, TS], F32, tag="acc")
                        for k in range(KC):
                            nc.tensor.matmul(
                                acc[:, :],
                                w_sb[:, k, f * 128 : (f + 1) * 128],
                                xT_sb[:, k, ts * TS : (ts + 1) * TS],
                                start=(k == 0),
                                stop=(k == KC - 1),
                            )
                        nc.vector.tensor_scalar_add(
                            dst[:, ts * TS : (ts + 1) * TS],
                            acc[:, :],
                            b_sb[:, f : f + 1],
                        )

                n_vdone = [0]

                def v_chain():
                    t = n_vdone[0]
                    n_vdone[0] += 1
                    acc = pp1.tile([128, GF], F32, tag="acc")
                    for k in range(KC):
                        nc.tensor.matmul(
                            acc[:, :],
                            xT_sb[:, k, t * 128 : (t + 1) * 128],
                            wv_sb[:, k, :],
                            start=(k == 0),
                            stop=(k == KC - 1),
                        )
                    nc.vector.tensor_tensor(
                        v_sb[:, t, :, 0:64],
                        acc[:, :].rearrange("p (h d) -> p h d", h=HPC),
                        bvb_sb[:, :].rearrange("p (h d) -> p h d", h=HPC),
                        op=mybir.AluOpType.add,
                    )

                alloc_qk(0)
                for f in range(4):
                    qp, kp = fqk[f]
                    if f < 3:
                        alloc_qk(f + 1)
                    # filler queue for this head-pair's attention phase:
                    # v chains (f=0; front so PV deps resolve), then next
                    # head-pair's q/k projection chains
                    fillers = []
                    if f == 0:
                        fillers += [("v", v_chain)] * NT
                    if f < 3:
                        fillers += [
                            ("qk", (lambda ff=f + 1, ts=ts: emit_qk(ff, ts)))
                            for ts in range(NTS)
                        ]
                    NB = sum(2 * t + 2 for t in range(NTS))
                    gi = [0]

                    def drain_to_target():
                        # keep pace: finish all fillers by the last batch
                        gi[0] += 1
                        remaining = max(NB - gi[0], 1)
                        n = (len(fillers) + remaining - 1) // remaining
                        for _ in range(min(n, len(fillers))):
                            fillers.pop(0)[1]()

                    def ensure_v(t2max):
                        # pop v-chain fillers (queue front) until key block
                        # t2max is projected; a PV emitted before its v chain
                        # would deadlock the in-order Tensor queue
                        while n_vdone[0] <= t2max:
                            kind, fn = fillers.pop(0)
                            assert kind == "v"
                            fn()

                    for t1i in range(NTS):
                        emit_qk(f, t1i)
                        t1s = t1i * TS
                        nfull = t1s // 128
                        o_ps = [
                            pso.tile([65, TS], F32, tag="outps", name=f"ops{r}")
                            for r in range(2)
                        ]
                        nmm = [0, 0]
                        batches = [("full", t2p) for t2p in range(nfull // 2)]
                        batches += [("diag", 0), ("diag", 1)]

                        def emit_scores(b, r):
                            kind, idx = b
                            s_ps = ps2.tile(
                                [128, 1024], F32, tag=f"sps{r}", name=f"sps{r}"
                            )
                            if kind == "full":
                                for j in range(2):
                                    t2 = 2 * idx + j
                                    nc.tensor.matmul(
                                        s_ps[:, j * TS : (j + 1) * TS],
                                        kp[r * 64 : (r + 1) * 64, t2 * 128 : (t2 + 1) * 128],
                                        qp[r * 64 : (r + 1) * 64, t1s : t1s + TS],
                                        start=True,
                                        stop=True,
                                    )
                            else:
                                ds = (0, 1) if idx == 0 else (2, 3)
                                offs = (0, TS) if idx == 0 else (0, 256)
                                for d, off in zip(ds, offs):
                                    t2 = nfull + d
                                    nd = TS - 128 * d
                                    nc.tensor.matmul(
                                        s_ps[:, off : off + nd],
                                        kp[r * 64 : (r + 1) * 64, t2 * 128 : (t2 + 1) * 128],
                                        qp[r * 64 : (r + 1) * 64, t1s + 128 * d : t1s + TS],
                                        start=True,
                                        stop=True,
                                    )
                            return s_ps

                        def emit_exp(b, r, s_ps):
                            kind, idx = b
                            E = pE.tile([128, 1024], BF16, tag="E", name=f"E{r}")
                            if kind == "full":
                                nc.scalar.activation(E[:, :], s_ps[:, :], AF.Exp)
                            else:
                                width = 896 if idx == 0 else 384
                                nc.scalar.activation(
                                    E[:, 0:width], s_ps[:, 0:width], AF.Exp
                                )
                                # causal mask: zero the upper triangle of the
                                # two diagonal 128x128 blocks post-exp (bf16
                                # on DVE, off the PSUM critical path)
                                for off in (0, TS) if idx == 0 else (0, 256):
                                    nc.vector.tensor_mul(
                                        E[:, off : off + 128],
                                        E[:, off : off + 128],
                                        tri_sb[:, :],
                                    )
                            return E

                        def emit_pv(b, r, E):
                            kind, idx = b
                            if kind == "full":
                                for j in range(2):
                                    t2 = 2 * idx + j
                                    nc.tensor.matmul(
                                        o_ps[r][:, :],
                                        v_sb[:, t2, 2 * f + r, :],
                                        E[:, j * TS : (j + 1) * TS],
                                        start=(nmm[r] == 0),
                                        stop=False,
                                    )
                                    nmm[r] += 1
                            else:
                                ds = (0, 1) if idx == 0 else (2, 3)
                                offs = (0, TS) if idx == 0 else (0, 256)
                                for d, off in zip(ds, offs):
                                    t2 = nfull + d
                                    nd = TS - 128 * d
                                    nc.tensor.matmul(
                                        o_ps[r][:, 128 * d : TS],
                                        v_sb[:, t2, 2 * f + r, :],
                                        E[:, off : off + nd],
                                        start=(nmm[r] == 0),
                                        stop=(d == 3),
                                    )
                                    nmm[r] += 1

                        def t2max(b):
                            kind, idx = b
                            if kind == "full":
                                return 2 * idx + 1
                            return nfull + (1 if idx == 0 else 3)

                        # software pipeline: exps first on Scalar; on the
                        # Tensor queue PVs (E inputs long ready) and filler
                        # projection chains go ahead of scores(i+1) so the
                        # queue never idles behind a scores matmul waiting
                        # for exp(i) to free its PSUM bank
                        sps_cur = [emit_scores(batches[0], r) for r in range(2)]
                        pvq = []
                        for i, b in enumerate(batches):
                            Es = [emit_exp(b, r, sps_cur[r]) for r in range(2)]
                            if len(pvq) >= 2:
                                pb, pEs = pvq.pop(0)
                                ensure_v(t2max(pb))
                                for r in range(2):
                                    emit_pv(pb, r, pEs[r])
                            sps_nxt = [None, None]
                            if i + 1 < len(batches):
                                sps_nxt = [
                                    emit_scores(batches[i + 1], r) for r in range(2)
                                ]
                            # fillers go BEHIND scores(i+1): scores re-enter
                            # the PSUM round-trip immediately; fillers only
                            # soak up whatever Tensor idle remains
                            drain_to_target()
                            pvq.append((b, Es))
                            sps_cur = sps_nxt
                        for pb, pEs in pvq:
                            ensure_v(t2max(pb))
                            for r in range(2):
                                emit_pv(pb, r, pEs[r])

                        # normalize rows 0-63 by row 64 (softmax denominator):
                        # reciprocal on the [1,512] row, then broadcast
                        for r in range(2):
                            denom = pN.tile([1, TS], F32, tag="dn", name=f"dn{r}")
                            nc.vector.tensor_copy(denom[:, :], o_ps[r][64:65, :])
                            bc_sb = pN.tile([64, TS], F32, tag="bc", name=f"bc{r}")
                            nc.gpsimd.partition_broadcast(bc_sb[:, :], denom[:, :])
                            rc_sb = pN.tile([64, TS], F32, tag="rc", name=f"rc{r}")
                            nc.vector.reciprocal_approx_fast(rc_sb[:, :], bc_sb[:, :])
                            nc.vector.tensor_mul(
                                aP[f][r * 64 : (r + 1) * 64, t1s : t1s + TS],
                                o_ps[r][0:64, :],
                                rc_sb[:, :],
                            )
                        if f == 3:
                            # all heads done for this t1 range: output
                            # projection chunks become fillers for the next
                            # slice's attention (last slice drains at the end)
                            def oproj_chunk(t, n):
                                acc3 = pp1.tile([128, TS], F32, tag="acc")
                                for ff in range(4):
                                    nc.tensor.matmul(
                                        acc3[:, :],
                                        aP[ff][:, t * 128 : (t + 1) * 128],
                                        wo_sb[:, ff, n * TS : (n + 1) * TS],
                                        start=(ff == 0),
                                        stop=(ff == 3),
                                    )
                                stg = p3s.tile([128, TS], F16, tag="stg3")
                                if (t + n) % 2 == 0:
                                    nc.vector.tensor_copy(stg[:, :], acc3[:, :])
                                else:
                                    nc.scalar.copy(stg[:, :], acc3[:, :])
                                nc.gpsimd.dma_start(
                                    out[
                                        t * 128 : (t + 1) * 128,
                                        n * TS : (n + 1) * TS,
                                    ],
                                    stg[:, :],
                                )

                            for t in range(t1s // 128, t1s // 128 + 4):
                                for n in range(2):
                                    fillers.append(
                                        ("op", (lambda tt=t, nn=n: oproj_chunk(tt, nn)))
                                    )

                    while fillers:
                        fillers.pop(0)[1]()

    nc.finalize()
    return nc


def make_in_maps(x, w_qkv, b_qkv, w_out, b_out):
    x = np.asarray(x, dtype=np.float32)
    w_qkv = np.asarray(w_qkv, dtype=np.float32)
    b_qkv = np.asarray(b_qkv, dtype=np.float32)
    w_out = np.asarray(w_out, dtype=np.float32)

    def bf(a):
        return np.ascontiguousarray(a).astype(BF16NP)

    scale = 1.0 / np.sqrt(D)
    # multiplicative causal mask for diagonal blocks, applied to exp(scores):
    # rows = keys, cols = queries; keep where query >= key (upper triangle)
    tri01 = np.triu(np.ones((128, 128), dtype=np.float32)).astype(BF16NP)
    in_maps = []
    for core in range(NCORES):
        b, g = core // 2, core % 2
        sl = slice(g * GF, (g + 1) * GF)
        bq = (b_qkv[sl] * scale).reshape(4, 128).T  # [128, 4] per-feat col
        bk = b_qkv[C + g * GF : C + (g + 1) * GF].reshape(4, 128).T
        in_maps.append(
            {
                "xT": bf(x[b].T),
                "wq": bf(w_qkv[:, sl] * scale),
                "wk": bf(w_qkv[:, C + g * GF : C + (g + 1) * GF]),
                "wv": bf(w_qkv[:, 2 * C + g * GF : 2 * C + (g + 1) * GF]),
                "wo": bf(w_out[sl, :]),
                "bqc": np.ascontiguousarray(bq, dtype=np.float32),
                "bkc": np.ascontiguousarray(bk, dtype=np.float32),
                "bv": bf(b_qkv[2 * C + g * GF : 2 * C + (g + 1) * GF].reshape(1, GF)),
                "tri01": tri01,
            }
        )
    return in_maps


_NC_CACHE = {}


def run(inputs: dict, trace: bool = False):
    """Compile (cached) + run on 8 cores. Returns (full_output, BassKernelResults)."""
    if "nc" not in _NC_CACHE:
        _NC_CACHE["nc"] = build_nc()
    nc = _NC_CACHE["nc"]
    in_maps = make_in_maps(**inputs)
    res = run_bass_kernel_spmd(
        nc, in_maps, core_ids=list(range(NCORES)), trace=trace
    )
    outs = [np.asarray(m["out"], dtype=np.float32) for m in res.results]
    full = np.stack([outs[2 * b] + outs[2 * b + 1] for b in range(B)], axis=0)
    full += np.asarray(inputs["b_out"], dtype=np.float32)
    return full, res


def kernel(**inputs) -> np.ndarray:
    full, _ = run(inputs, trace=False)
    return full



# revision 75
# speedup vs baseline: 1.1160x; 1.0601x over previous
"""Causal self-attention (B=4, T=2048, C=1024, H=16, D=64) on 8 TRN2 cores.

Sharding: core i = (batch b=i//2, head-group g=i%2 of 8 heads).
Each core runs the full pipeline for its (b, g) shard with zero
cross-core communication; the row-parallel out_proj partial sums of the
two head-groups of a batch are added on the host during unsharding
(the device emits fp16 partials; the host adds them in fp32 + b_out).

Compute dtype: bfloat16 operands, fp32 PSUM accumulation (full PE rate).

Per-core dataflow (single fused software-pipelined stream):
  - DMA loads split over two trigger queues (Sync: wq+xT; Pool: wk,
    consts, wv, wo) in f=0 critical-path-first order.
  - Per head-pair f (heads 2f, 2f+1 on partition halves of qp/kp):
    attention runs as a flat stream of single 128-key blocks across the
    four 512-query t1 slices.  Per block: 2 score matmuls (scores^T
    [t2,t1], K=64, one per head, each at its own PSUM bank of a shared
    [128,1024] sps tile, tag-rotated over 2 buffers so exp(j) reading
    one buffer never blocks scores(j+1) writing the other), one EXP ACT
    on ScalarE covering both heads via a strided 2-segment read (no max
    subtraction -- scores are O(1)), post-exp causal masking of diag
    128x128 blocks on DVE, and 2 PV matmuls with lhsT=[v_h | ones] so
    PSUM row 64 accumulates the softmax denominator.  Normalization
    (reciprocal+broadcast+mul) drains to SBUF per slice.
  - The Tensor queue is kept saturated by interleaved filler chains:
    v projection (during f=0), q/k projection of head-pair f+1 (during
    f<3), and out_proj chunks of the previous slice (during f=3).
"""

import os
import sys

for _p in (
    "/root/.axon_site",
    "/root/.axon_site/_ro/trn_rl_repo",
    "/root/.axon_site/_ro/pypackages",
    "/opt/trn_rl_repo",
):
    if os.path.isdir(_p) and _p not in sys.path:
        sys.path.append(_p)

import numpy as np
import ml_dtypes

import concourse.bass as bass
import concourse.bacc as bacc
import concourse.mybir as mybir
from concourse import tile
from concourse.bass_utils import run_bass_kernel_spmd

BF16NP = ml_dtypes.bfloat16

B, T, C, H, D = 4, 2048, 1024, 16, 64
HPC = 8            # heads per core
GF = HPC * D       # 512: feature width of one head-group
NCORES = 8
KC = C // 128      # 8 contraction tiles over C
NT = T // 128      # 16 time tiles of 128
TS = 512           # t1 slice width
NTS = T // TS      # 4 t1 slices

F32 = mybir.dt.float32
F16 = mybir.dt.float16
BF16 = mybir.dt.bfloat16
AF = mybir.ActivationFunctionType


def build_nc() -> bass.Bass:
    nc = bacc.Bacc()

    xT = nc.declare_dram_parameter("xT", [C, T], BF16, isOutput=False)
    wq = nc.declare_dram_parameter("wq", [C, GF], BF16, isOutput=False)
    wk = nc.declare_dram_parameter("wk", [C, GF], BF16, isOutput=False)
    wv = nc.declare_dram_parameter("wv", [C, GF], BF16, isOutput=False)
    wo = nc.declare_dram_parameter("wo", [GF, C], BF16, isOutput=False)
    bqc = nc.declare_dram_parameter("bqc", [128, 4], F32, isOutput=False)
    bkc = nc.declare_dram_parameter("bkc", [128, 4], F32, isOutput=False)
    bv = nc.declare_dram_parameter("bv", [1, GF], BF16, isOutput=False)
    tri01 = nc.declare_dram_parameter("tri01", [128, 128], BF16, isOutput=False)
    out = nc.declare_dram_parameter("out", [T, C], F16, isOutput=True)

    with tile.TileContext(nc) as tc:
        with (
            tc.tile_pool(name="dram", bufs=1, space="DRAM") as dpool,
            tc.tile_pool(name="consts", bufs=1) as cpool,
            tc.tile_pool(name="apool", bufs=1) as apool,
        ):
            # v resident in SBUF: [t2-part, t2-tile, head, 64 v + 1 ones]
            v_sb = apool.tile([128, NT, HPC, 65], BF16, tag="vsb")
            nc.vector.memset(v_sb[:, :, :, 64:65], 1.0)
            # attention outputs stay resident in SBUF, feature-major
            aP = [apool.tile([128, T], BF16, tag=f"aP{f}", name=f"aP{f}") for f in range(4)]

            # const loads go on the Pool trigger queue, after wk (see below):
            # the Sync queue must reach the f=0 critical-path loads first
            tri_sb = cpool.tile([128, 128], BF16)
            bqc_sb = cpool.tile([128, 4], F32, tag="bqc")
            bkc_sb = cpool.tile([128, 4], F32, tag="bkc")
            bv_sb = cpool.tile([1, GF], BF16, tag="bv")
            bvb_sb = cpool.tile([128, GF], BF16, tag="bvb")

            # ------- phases 1+2 interleaved: qkv projection + attention -------
            with (
                tc.tile_pool(name="p1", bufs=1) as p1,
                tc.tile_pool(name="p1s", bufs=16) as p1s,
                tc.tile_pool(name="p2", bufs=2) as p2,
                tc.tile_pool(name="pE", bufs=16) as pE,
                tc.tile_pool(name="pN", bufs=2) as pN,
                tc.tile_pool(name="p3s", bufs=6) as p3s,
                tc.tile_pool(name="pp1", bufs=2, space="PSUM") as pp1,
                tc.tile_pool(name="ps2", bufs=2, space="PSUM") as ps2,
                tc.tile_pool(name="pso", bufs=2, space="PSUM") as pso,
            ):
                # DMA priority order over two trigger queues (Sync + Pool):
                # the f=0 q/k projection's inputs first, wo last.
                xT_sb = p1.tile([128, KC, T], BF16)
                wq_sb = p1.tile([128, KC, GF], BF16, tag="wq")
                wk_sb = p1.tile([128, KC, GF], BF16, tag="wk")
                wv_sb = p1.tile([128, KC, GF], BF16, tag="wv")
                wo_sb = p1.tile([128, 4, C], BF16, tag="wo")
                # xT lands in two passes: cols 0:512 first (all the f=0
                # ts=0 q/k chains need) so attention starts ~6 us earlier,
                # then the remaining cols
                for k in range(KC):
                    ksl = slice(k * 128, (k + 1) * 128)
                    nc.sync.dma_start(wq_sb[:, k, :], wq[ksl, :])
                    nc.sync.dma_start(xT_sb[:, k, 0:TS], xT[ksl, 0:TS])
                for k in range(KC):
                    ksl = slice(k * 128, (k + 1) * 128)
                    nc.sync.dma_start(xT_sb[:, k, TS : 2 * TS], xT[ksl, TS : 2 * TS])
                for k in range(KC):
                    ksl = slice(k * 128, (k + 1) * 128)
                    nc.sync.dma_start(xT_sb[:, k, 2 * TS : T], xT[ksl, 2 * TS : T])
                for k in range(KC):
                    ksl = slice(k * 128, (k + 1) * 128)
                    nc.gpsimd.dma_start(wk_sb[:, k, :], wk[ksl, :])
                nc.gpsimd.dma_start(bqc_sb[:, :], bqc[:, :])
                nc.gpsimd.dma_start(bkc_sb[:, :], bkc[:, :])
                nc.gpsimd.dma_start(bv_sb[:, :], bv[:, :])
                nc.gpsimd.dma_start(tri_sb[:, :], tri01[:, :])
                for k in range(KC):
                    ksl = slice(k * 128, (k + 1) * 128)
                    nc.gpsimd.dma_start(wv_sb[:, k, :], wv[ksl, :])
                # bias of v broadcast to all partitions once; folded into the
                # PSUM->SBUF copy of v (replaces the ones/bv bias matmul).
                # Emitted after the wv triggers: it blocks the Pool queue
                # until bv lands, which must not delay wv (v chains start
                # ~15 us in and were observed stalling ~7 us on late wv).
                nc.gpsimd.partition_broadcast(bvb_sb[:, :], bv_sb[:, :])
                nc.gpsimd.dma_start(
                    wo_sb[:, :, :], wo[:, :].rearrange("(n p) c -> p n c", p=128)
                )

                # ---- projection chain emitters (each ~8 matmuls of filler
                # work for the Tensor queue, used to absorb exp-wait stalls
                # in the Scalar-bound attention phases) ----
                fqk = {}

                def alloc_qk(f):
                    fqk[f] = (
                        p2.tile([128, T], BF16, tag="qp", name=f"qp{f}"),
                        p2.tile([128, T], BF16, tag="kp", name=f"kp{f}"),
                    )

                qk_done = set()

                def emit_qk(f, ts):
                    # q+k projection chains for head-pair f, t1-slice ts
                    if (f, ts) in qk_done:
                        return
                    qk_done.add((f, ts))
                    qpf, kpf = fqk[f]
                    for w_sb, b_sb, dst in (
                        (wq_sb, bqc_sb, qpf),
                        (wk_sb, bkc_sb, kpf),
                    ):
                        acc = pp1.tile([128, TS], F32, tag="acc")
                        for k in range(KC):
                            nc.tensor.matmul(
                                acc[:, :],
                                w_sb[:, k, f * 128 : (f + 1) * 128],
                                xT_sb[:, k, ts * TS : (ts + 1) * TS],
                                start=(k == 0),
                                stop=(k == KC - 1),
                            )
                        nc.vector.tensor_scalar_add(
                            dst[:, ts * TS : (ts + 1) * TS],
                            acc[:, :],
                            b_sb[:, f : f + 1],
                        )

                n_vdone = [0]

                def v_chain():
                    t = n_vdone[0]
                    n_vdone[0] += 1
                    acc = pp1.tile([128, GF], F32, tag="acc")
                    for k in range(KC):
                        nc.tensor.matmul(
                            acc[:, :],
                            xT_sb[:, k, t * 128 : (t + 1) * 128],
                            wv_sb[:, k, :],
                            start=(k == 0),
                            stop=(k == KC - 1),
                        )
                    nc.vector.tensor_tensor(
                        v_sb[:, t, :, 0:64],
                        acc[:, :].rearrange("p (h d) -> p h d", h=HPC),
                        bvb_sb[:, :].rearrange("p (h d) -> p h d", h=HPC),
                        op=mybir.AluOpType.add,
                    )

                alloc_qk(0)
                for f in range(4):
                    qp, kp = fqk[f]
                    if f < 3:
                        alloc_qk(f + 1)
                    # filler queue for this head-pair's attention phase:
                    # v chains (f=0; front so PV deps resolve), then next
                    # head-pair's q/k projection chains
                    fillers = []
                    if f == 0:
                        fillers += [("v", v_chain)] * NT
                    if f < 3:
                        fillers += [
                            ("qk", (lambda ff=f + 1, ts=ts: emit_qk(ff, ts)))
                            for ts in range(NTS)
                        ]
                    NB = sum(4 * t + 4 for t in range(NTS))
                    gi = [0]

                    def drain_to_target():
                        # keep pace: finish all fillers by the last batch
                        gi[0] += 1
                        remaining = max(NB - gi[0], 1)
                        n = (len(fillers) + remaining - 1) // remaining
                        for _ in range(min(n, len(fillers))):
                            fillers.pop(0)[1]()

                    def ensure_v(t2max):
                        # pop v-chain fillers (queue front) until key block
                        # t2max is projected; a PV emitted before its v chain
                        # would deadlock the in-order Tensor queue
                        while n_vdone[0] <= t2max:
                            kind, fn = fillers.pop(0)
                            assert kind == "v"
                            fn()

                    # flat stream of single key blocks across all four t1
                    # slices; both heads (r0/r1) share one sps tile (cols
                    # 0:512 / 512:1024) so one tag with bufs=2 gives true
                    # double buffering: scores(j+1) write one buffer while
                    # exp(j) reads the other -- no exp->scores round trip.
                    # Cross-slice flow also kills slice-boundary bubbles.
                    allb = []
                    for t1i in range(NTS):
                        allb += [
                            (t1i, t2, max(t2 - 4 * t1i, -1))
                            for t2 in range(4 * t1i + 4)
                        ]
                    assert len(allb) == NB

                    o_ps_of = {}
                    nmm_of = {}

                    def get_ops(t1i):
                        if t1i not in o_ps_of:
                            o_ps_of[t1i] = [
                                pso.tile([65, TS], F32, tag="outps", name=f"ops{r}")
                                for r in range(2)
                            ]
                            nmm_of[t1i] = [0, 0]
                        return o_ps_of[t1i]

                    def emit_scores(bt):
                        t1i, t2, d = bt
                        t1s = t1i * TS
                        nd = TS if d < 0 else TS - 128 * d
                        s_ps = ps2.tile([128, 1024], F32, tag="sps", name="sps")
                        for r in range(2):
                            # segment per head starts at its own PSUM bank (a
                            # matmul output may not straddle a bank boundary)
                            nc.tensor.matmul(
                                s_ps[:, r * TS : r * TS + nd],
                                kp[r * 64 : (r + 1) * 64, t2 * 128 : (t2 + 1) * 128],
                                qp[r * 64 : (r + 1) * 64, t1s + TS - nd : t1s + TS],
                                start=True,
                                stop=True,
                            )
                        return s_ps

                    def emit_exp(bt, s_ps):
                        t1i, t2, d = bt
                        nd = TS if d < 0 else TS - 128 * d
                        E = pE.tile([128, 1024], BF16, tag="E", name="E")
                        # one ACT covers both heads: strided 2-segment read
                        # from PSUM (segments at bank starts), contiguous
                        # packed write to E
                        nc.scalar.activation(
                            E[:, 0 : 2 * nd].rearrange("p (s x) -> p s x", s=2),
                            s_ps[:, :].rearrange("p (s x) -> p s x", s=2)[:, :, 0:nd],
                            AF.Exp,
                        )
                        if d >= 0:
                            # causal mask: zero the upper triangle of the
                            # diagonal 128x128 block of each head, post-exp
                            # (bf16 on DVE, off the PSUM critical path)
                            for off in (0, nd):
                                nc.vector.tensor_mul(
                                    E[:, off : off + 128],
                                    E[:, off : off + 128],
                                    tri_sb[:, :],
                                )
                        return E

                    def emit_pv(bt, r, E):
                        t1i, t2, d = bt
                        nd = TS if d < 0 else TS - 128 * d
                        o_ps = get_ops(t1i)
                        nmm = nmm_of[t1i]
                        nc.tensor.matmul(
                            o_ps[r][:, TS - nd : TS],
                            v_sb[:, t2, 2 * f + r, :],
                            E[:, r * nd : (r + 1) * nd],
                            start=(nmm[r] == 0),
                            stop=(nmm[r] == 4 * t1i + 3),
                        )
                        nmm[r] += 1

                    def t2max(bt):
                        return bt[1]

                    def oproj_chunk(t, n):
                        acc3 = pp1.tile([128, TS], F32, tag="acc")
                        for ff in range(4):
                            nc.tensor.matmul(
                                acc3[:, :],
                                aP[ff][:, t * 128 : (t + 1) * 128],
                                wo_sb[:, ff, n * TS : (n + 1) * TS],
                                start=(ff == 0),
                                stop=(ff == 3),
                            )
                        stg = p3s.tile([128, TS], F16, tag="stg3")
                        nc.vector.tensor_copy(stg[:, :], acc3[:, :])
                        nc.sync.dma_start(
                            out[t * 128 : (t + 1) * 128, n * TS : (n + 1) * TS],
                            stg[:, :],
                        )

                    def finish_slice(t1i):
                        # normalize rows 0-63 by row 64 (softmax denominator)
                        t1s = t1i * TS
                        o_ps = o_ps_of[t1i]
                        for r in range(2):
                            denom = pN.tile([1, TS], F32, tag="dn", name=f"dn{r}")
                            nc.vector.tensor_copy(denom[:, :], o_ps[r][64:65, :])
                            rd_sb = pN.tile([1, TS], F32, tag="rd", name=f"rd{r}")
                            nc.vector.reciprocal_approx_fast(rd_sb[:, :], denom[:, :])
                            rc_sb = pN.tile([64, TS], F32, tag="rc", name=f"rc{r}")
                            nc.gpsimd.partition_broadcast(rc_sb[:, :], rd_sb[:, :])
                            nc.vector.tensor_mul(
                                aP[f][r * 64 : (r + 1) * 64, t1s : t1s + TS],
                                o_ps[r][0:64, :],
                                rc_sb[:, :],
                            )
                        if f == 3:
                            # out_proj chunks become fillers for the next
                            # slice's attention (last slice drains at the end)
                            for t in range(4 * t1i, 4 * t1i + 4):
                                for n in range(2):
                                    fillers.append(
                                        ("op", (lambda tt=t, nn=n: oproj_chunk(tt, nn)))
                                    )

                    def pv_and_finish(pb, pE_):
                        ensure_v(t2max(pb))
                        for r in range(2):
                            emit_pv(pb, r, pE_)
                        t1i = pb[0]
                        if nmm_of[t1i] == [4 * t1i + 4, 4 * t1i + 4]:
                            finish_slice(t1i)

                    emit_qk(f, 0)
                    sps_cur = emit_scores(allb[0])
                    pvq = []
                    for i, bt in enumerate(allb):
                        E = emit_exp(bt, sps_cur)
                        # scores(i+1) lead the Tensor queue so they complete
                        # before exp(i+1) needs them; PVs and fillers follow
                        sps_nxt = None
                        if i + 1 < NB:
                            nxt = allb[i + 1]
                            if nxt[0] != bt[0]:
                                emit_qk(f, nxt[0])
                            sps_nxt = emit_scores(nxt)
                        # shallower PV pipeline near the stream end: the
                        # final slice's normalization (and f=3's last
                        # out_proj chunks) start sooner
                        while len(pvq) >= (4 if i < NB - 3 else 1):
                            pv_and_finish(*pvq.pop(0))
                        drain_to_target()
                        pvq.append((bt, E))
                        sps_cur = sps_nxt
                    for pb_pE in pvq:
                        pv_and_finish(*pb_pE)

                    while fillers:
                        fillers.pop(0)[1]()

    nc.finalize()
    return nc


def make_in_maps(x, w_qkv, b_qkv, w_out, b_out):
    x = np.asarray(x, dtype=np.float32)
    w_qkv = np.asarray(w_qkv, dtype=np.float32)
    b_qkv = np.asarray(b_qkv, dtype=np.float32)
    w_out = np.asarray(w_out, dtype=np.float32)

    def bf(a):
        return np.ascontiguousarray(a).astype(BF16NP)

    scale = 1.0 / np.sqrt(D)
    # multiplicative causal mask for diagonal blocks, applied to exp(scores):
    # rows = keys, cols = queries; keep where query >= key (upper triangle)
    tri01 = np.triu(np.ones((128, 128), dtype=np.float32)).astype(BF16NP)
    in_maps = []
    for core in range(NCORES):
        b, g = core // 2, core % 2
        sl = slice(g * GF, (g + 1) * GF)
        bq = (b_qkv[sl] * scale).reshape(4, 128).T  # [128, 4] per-feat col
        bk = b_qkv[C + g * GF : C + (g + 1) * GF].reshape(4, 128).T
        in_maps.append(
            {
                "xT": bf(x[b].T),
                "wq": bf(w_qkv[:, sl] * scale),
                "wk": bf(w_qkv[:, C + g * GF : C + (g + 1) * GF]),
                "wv": bf(w_qkv[:, 2 * C + g * GF : 2 * C + (g + 1) * GF]),
                "wo": bf(w_out[sl, :]),
                "bqc": np.ascontiguousarray(bq, dtype=np.float32),
                "bkc": np.ascontiguousarray(bk, dtype=np.float32),
                "bv": bf(b_qkv[2 * C + g * GF : 2 * C + (g + 1) * GF].reshape(1, GF)),
                "tri01": tri01,
            }
        )
    return in_maps


_NC_CACHE = {}


def run(inputs: dict, trace: bool = False):
    """Compile (cached) + run on 8 cores. Returns (full_output, BassKernelResults)."""
    if "nc" not in _NC_CACHE:
        _NC_CACHE["nc"] = build_nc()
    nc = _NC_CACHE["nc"]
    in_maps = make_in_maps(**inputs)
    res = run_bass_kernel_spmd(
        nc, in_maps, core_ids=list(range(NCORES)), trace=trace
    )
    outs = [np.asarray(m["out"], dtype=np.float32) for m in res.results]
    full = np.stack([outs[2 * b] + outs[2 * b + 1] for b in range(B)], axis=0)
    full += np.asarray(inputs["b_out"], dtype=np.float32)
    return full, res


def kernel(**inputs) -> np.ndarray:
    full, _ = run(inputs, trace=False)
    return full

